# revision 15
# baseline (speedup 1.0000x reference)
"""Trainium2 Bass kernel for EnhancedBiologicalSplatAttentionLayer.

Reference computation (B=4, S=2048, D=1024, K=64):
    v    = x @ Wv.T                                   [B,S,D]
    aff  = normalize_k(exp(-0.5*dist_sq(x, centers)/scale^2))   [B,S,K]
    st   = aff.T @ v   (per batch)                    [B,K,D]
    tok  = aff @ st                                   [B,S,D]
    out  = tok @ Wo.T                                 [B,S,D]

Algebraic reduction used here (exact reassociation):
    M = aff.T @ x            [K,D]   (per batch)
    out = aff @ ((M @ Wv.T) @ Wo.T)
which avoids both [S,D]x[D,D] projections over the full sequence
(37.7 GFLOP -> ~4.3 GFLOP).

Sharding over 8 cores, no cross-core communication:
    core c -> batch b = c//2, output-dim half j = c%2.
    Each core computes the full affinity pipeline + splat summary M for its
    batch (duplicated within the pair), and produces out[b][:, j*512:(j+1)*512].

Matmul operands are bf16 (1 cyc/row on PE at any free size); accumulation in
PSUM fp32; affinity/normalization arithmetic fp32. The exp() input here is
~-500, which underflows to exactly 0.0 in fp32 — faithfully matching the
reference numerics (the reference also underflows; no softmax-max-trick).
"""
import numpy as np
import ml_dtypes

import concourse.bass as bass
import concourse.bacc as bacc
import concourse.tile as tile
from concourse import mybir
from concourse.masks import make_identity
from concourse.bass_utils import run_bass_kernel_spmd

B, S, D, K = 4, 2048, 1024, 64
P = 128
ST = S // P          # 16 s-tiles
DT = D // P          # 8 d-tiles
HALF = D // 2        # 512 output-dim half per core
EPS = 1e-8

BF = mybir.dt.bfloat16
F32 = mybir.dt.float32
BF_NP = ml_dtypes.bfloat16

_CACHE = {}


def _emit_zero_out(nc, tc, out_d):
    with tc.tile_pool(name="zo", bufs=2) as zo:
        for st in range(ST):
            o_sb = zo.tile([P, HALF], F32, tag="o_sb")
            nc.vector.memset(o_sb[:], 0.0)
            nc.sync.dma_start(
                out=out_d.ap()[st * P:(st + 1) * P, :], in_=o_sb[:],
            )


def build_nc(phase="full"):
    """phase: 'dma' (loads + zero out), 'a' (phase A + zero out), 'full'."""
    nc = bacc.Bacc("TRN2", target_bir_lowering=False, debug=False)

    xn_d = nc.dram_tensor("xn", [S, D], BF, kind="ExternalInput")
    xt_d = nc.dram_tensor("xt", [D, S], BF, kind="ExternalInput")
    cts_d = nc.dram_tensor("cts", [D, K], BF, kind="ExternalInput")
    brow_d = nc.dram_tensor("brow", [1, K], BF, kind="ExternalInput")
    grow_d = nc.dram_tensor("grow", [1, K], F32, kind="ExternalInput")
    wvt_d = nc.dram_tensor("wvt", [D, D], BF, kind="ExternalInput")
    wot_d = nc.dram_tensor("wot", [D, HALF], BF, kind="ExternalInput")
    out_d = nc.dram_tensor("out", [S, HALF], F32, kind="ExternalOutput")

    with tile.TileContext(nc) as tc:
        with tc.tile_pool(name="persist", bufs=1) as persist:
            # ---- persistent SBUF tensors -------------------------------
            ident = persist.tile([P, P], BF)
            make_identity(nc, ident)
            ones1 = persist.tile([1, P], BF)
            nc.vector.memset(ones1[:], 1.0)

            cts_sb = persist.tile([P, DT, K], BF)
            for dt in range(DT):
                nc.sync.dma_start(
                    out=cts_sb[:, dt, :], in_=cts_d.ap()[dt * P:(dt + 1) * P, :]
                )
            brow_sb = persist.tile([1, K], BF)
            nc.sync.dma_start(out=brow_sb[:], in_=brow_d.ap())
            g64 = persist.tile([P, K], F32)
            nc.sync.dma_start(out=g64[:], in_=grow_d.ap()[0].partition_broadcast(P))

            # x in both layouts, fully resident
            xt_sb = persist.tile([P, DT, S], BF)
            for dt in range(DT):
                nc.sync.dma_start(
                    out=xt_sb[:, dt, :], in_=xt_d.ap()[dt * P:(dt + 1) * P, :]
                )
            xn_sb = persist.tile([P, ST, D], BF)
            for st in range(ST):
                nc.sync.dma_start(
                    out=xn_sb[:, st, :], in_=xn_d.ap()[st * P:(st + 1) * P, :]
                )

            # weights, fully resident
            wvt_sb = persist.tile([P, DT, D], BF)
            for dt in range(DT):
                nc.sync.dma_start(
                    out=wvt_sb[:, dt, :], in_=wvt_d.ap()[dt * P:(dt + 1) * P, :]
                )
            wot_sb = persist.tile([P, DT, HALF], BF)
            for dt in range(DT):
                nc.sync.dma_start(
                    out=wot_sb[:, dt, :], in_=wot_d.ap()[dt * P:(dt + 1) * P, :]
                )

            # affinities in both layouts
            A_sk = persist.tile([P, ST, K], BF)     # [s-part, st, k]
            A_ks = persist.tile([K, ST, P], BF)     # [k-part, st, s]
            m_sb = persist.tile([K, D], BF)

            # ---- phase A: affinities + M -------------------------------
            # sub-levels for bisection: a1=xsq only, a2=+xc+exp, a3=+norm,
            # a4=+transpose, a5/a=everything
            lvl = {"a1": 1, "a2": 2, "a3": 3, "a4": 4, "a5": 5,
                   "a": 5, "full": 5}.get(phase, 0)
            if lvl >= 1:
                with (
                    tc.tile_pool(name="ps_M_pool", bufs=1, space="PSUM") as ps_M_pool,
                    tc.tile_pool(name="pa_sc", bufs=3) as pa_sc,
                    tc.tile_pool(name="pa_sm", bufs=4) as pa_sm,
                    tc.tile_pool(name="pa_ps", bufs=2, space="PSUM") as pa_ps,
                    tc.tile_pool(name="pa_pstr", bufs=2, space="PSUM") as pa_pstr,
                ):
                    ps_M = ps_M_pool.tile([K, D], F32)  # splat summary accum
                    for st in range(ST):
                        xn_t = xn_sb[:, st, :]
                        # x_sq (fused square+rowsum, DVE)
                        sq_scratch = pa_sc.tile([P, D], F32, tag="sq")
                        xsq = pa_sm.tile([P, 1], F32, tag="xsq")
                        nc.vector.tensor_mul(sq_scratch[:], xn_t, xn_t)
                        nc.vector.tensor_reduce(
                            out=xsq[:], in_=sq_scratch[:],
                            axis=mybir.AxisListType.X, op=mybir.AluOpType.add,
                        )
                        # xsqg[s,k] = -0.5*inv_ss[k] * x_sq[s]
                        xsqg = pa_sm.tile([P, K], F32, tag="xsqg")
                        nc.vector.tensor_scalar_mul(xsqg[:], g64[:], xsq[:])
                        if lvl < 2:
                            continue

                        # xc_scaled + c_sq term via PE accumulation
                        ps_t = pa_ps.tile([P, K], F32, tag="ps_t")
                        for dt in range(DT):
                            nc.tensor.matmul(
                                ps_t[:],
                                xt_sb[:, dt, st * P:(st + 1) * P],
                                cts_sb[:, dt, :],
                                start=(dt == 0), stop=False,
                            )
                        nc.tensor.matmul(
                            ps_t[:], ones1[:], brow_sb[:], start=False, stop=True,
                        )

                        # exp input assembly + exp
                        t2 = pa_sm.tile([P, K], F32, tag="t2")
                        nc.vector.tensor_add(t2[:], ps_t[:], xsqg[:])
                        au = pa_sm.tile([P, K], F32, tag="au")
                        nc.scalar.activation(
                            out=au[:], in_=t2[:],
                            func=mybir.ActivationFunctionType.Exp,
                        )
                        if lvl < 3:
                            continue
                        # normalize over k
                        den = pa_sm.tile([P, 1], F32, tag="den")
                        nc.vector.tensor_reduce(
                            out=den[:], in_=au[:], axis=mybir.AxisListType.X,
                            op=mybir.AluOpType.add,
                        )
                        nc.vector.tensor_scalar_add(den[:], den[:], EPS)
                        rec = pa_sm.tile([P, 1], F32, tag="rec")
                        nc.vector.reciprocal(out=rec[:], in_=den[:])
                        nc.vector.tensor_scalar_mul(A_sk[:, st, :], au[:], rec[:])

                        if lvl < 4:
                            continue
                        # A^T slice for the out-phase
                        ps_tr = pa_pstr.tile([K, P], BF, tag="ps_tr")
                        nc.tensor.transpose(ps_tr[:], A_sk[:, st, :], ident[:])
                        nc.any.tensor_copy(out=A_ks[:, st, :], in_=ps_tr[:])

                        if lvl < 5:
                            continue
                        # M accumulation: M += A_sk[st].T @ x[st]
                        for h in range(2):
                            nc.tensor.matmul(
                                ps_M[:, h * 512:(h + 1) * 512],
                                A_sk[:, st, :],
                                xn_sb[:, st, h * 512:(h + 1) * 512],
                                start=(st == 0), stop=(st == ST - 1),
                            )

                    # stash M to SBUF before the PSUM pool closes
                    if lvl >= 5:
                        nc.any.tensor_copy(out=m_sb[:], in_=ps_M[:])

            # ---- phase B: weight chain + output ------------------------
            if phase == "full":
                with (
                    tc.tile_pool(name="pb_sb", bufs=1) as pb_sb,
                    tc.tile_pool(name="pb_ps", bufs=1, space="PSUM") as pb_ps,
                    tc.tile_pool(name="pb_ptr", bufs=2, space="PSUM") as pb_ptr,
                    tc.tile_pool(name="pb_out", bufs=3) as pb_out,
                    tc.tile_pool(name="pb_pso", bufs=2, space="PSUM") as pb_pso,
                ):
                    # M^T tiles
                    mt_sb = persist.tile([P, DT, K], BF)
                    for dt in range(DT):
                        ps_mt = pb_ptr.tile([P, K], BF, tag="ps_mt")
                        nc.tensor.transpose(
                            ps_mt[:], m_sb[:, dt * P:(dt + 1) * P], ident[:K, :K],
                        )
                        nc.any.tensor_copy(out=mt_sb[:, dt, :], in_=ps_mt[:])

                    # N = M @ Wv.T
                    ps_N = pb_ps.tile([K, D], F32, tag="ps_N")
                    for dt in range(DT):
                        for h in range(2):
                            nc.tensor.matmul(
                                ps_N[:, h * 512:(h + 1) * 512],
                                mt_sb[:, dt, :],
                                wvt_sb[:, dt, h * 512:(h + 1) * 512],
                                start=(dt == 0), stop=(dt == DT - 1),
                            )
                    n_sb = pb_sb.tile([K, D], BF, tag="n_sb")
                    nc.any.tensor_copy(out=n_sb[:], in_=ps_N[:])

                    # N^T tiles
                    nt_sb = persist.tile([P, DT, K], BF)
                    for et in range(DT):
                        ps_nt = pb_ptr.tile([P, K], BF, tag="ps_mt")
                        nc.tensor.transpose(
                            ps_nt[:], n_sb[:, et * P:(et + 1) * P], ident[:K, :K],
                        )
                        nc.any.tensor_copy(out=nt_sb[:, et, :], in_=ps_nt[:])

                    # P = N @ Wo_half.T
                    ps_P = pb_ps.tile([K, HALF], F32, tag="ps_P")
                    for et in range(DT):
                        nc.tensor.matmul(
                            ps_P[:], nt_sb[:, et, :], wot_sb[:, et, :],
                            start=(et == 0), stop=(et == DT - 1),
                        )
                    p_sb = pb_sb.tile([K, HALF], BF, tag="p_sb")
                    nc.any.tensor_copy(out=p_sb[:], in_=ps_P[:])

                    # out[st] = A[st] @ P
                    for st in range(ST):
                        ps_o = pb_pso.tile([P, HALF], F32, tag="ps_o")
                        nc.tensor.matmul(
                            ps_o[:], A_ks[:, st, :], p_sb[:], start=True, stop=True,
                        )
                        o_sb = pb_out.tile([P, HALF], F32, tag="o_sb")
                        nc.any.tensor_copy(out=o_sb[:], in_=ps_o[:])
                        nc.sync.dma_start(
                            out=out_d.ap()[st * P:(st + 1) * P, :], in_=o_sb[:],
                        )
            else:
                _emit_zero_out(nc, tc, out_d)

    nc.compile()
    return nc


def _host_prep(x, splat_centers, splat_log_scales, w_value, w_out):
    """Fold scales into weights; build per-core input maps."""
    x = np.asarray(x, dtype=np.float32)
    centers = np.asarray(splat_centers, dtype=np.float32)
    log_scales = np.asarray(splat_log_scales, dtype=np.float32)
    w_value = np.asarray(w_value, dtype=np.float32)
    w_out = np.asarray(w_out, dtype=np.float32)

    scales = np.clip(np.exp(log_scales), 0.1, 2.0)
    inv_ss = (1.0 / (scales * scales)).astype(np.float32)          # [K]
    cts = (centers.T * inv_ss[None, :]).astype(BF_NP)              # [D,K]
    c_sq = (centers * centers).sum(axis=1).astype(np.float32)      # [K]
    brow = (-0.5 * c_sq * inv_ss)[None, :].astype(BF_NP)           # [1,K]
    grow = (-0.5 * inv_ss)[None, :].astype(np.float32)             # [1,K]
    wvt = w_value.T.astype(BF_NP).copy()                           # [D,D]

    in_maps = []
    for c in range(8):
        b, j = divmod(c, 2)
        xb = x[b]
        in_maps.append({
            "xn": xb.astype(BF_NP),
            "xt": xb.T.astype(BF_NP).copy(),
            "cts": cts,
            "brow": brow,
            "grow": grow,
            "wvt": wvt,
            "wot": w_out[j * HALF:(j + 1) * HALF, :].T.astype(BF_NP).copy(),
        })
    return in_maps


def run_on_hw(in_maps, trace=False, phase="full"):
    key = f"nc_{phase}"
    if key not in _CACHE:
        _CACHE[key] = build_nc(phase)
    return run_bass_kernel_spmd(_CACHE[key], in_maps, list(range(8)), trace=trace)


def kernel(**inputs) -> np.ndarray:
    in_maps = _host_prep(**inputs)
    res = run_on_hw(in_maps)
    out = np.empty((B, S, D), dtype=np.float32)
    for c in range(8):
        b, j = divmod(c, 2)
        out[b][:, j * HALF:(j + 1) * HALF] = res.results[c]["out"]
    return out


# revision 18
# speedup vs baseline: 27439.9002x; 27439.9002x over previous
"""Trainium2 Bass kernel for EnhancedBiologicalSplatAttentionLayer.

Reference computation (B=4, S=2048, D=1024, K=64):
    v    = x @ Wv.T                                   [B,S,D]
    aff  = normalize_k(exp(-0.5*dist_sq(x, centers)/scale^2))   [B,S,K]
    st   = aff.T @ v   (per batch)                    [B,K,D]
    tok  = aff @ st                                   [B,S,D]
    out  = tok @ Wo.T                                 [B,S,D]

Algebraic reduction used here (exact reassociation):
    M = aff.T @ x            [K,D]   (per batch)
    out = aff @ ((M @ Wv.T) @ Wo.T)
which avoids both [S,D]x[D,D] projections over the full sequence
(37.7 GFLOP -> ~4.3 GFLOP).

Sharding over 8 cores, no cross-core communication:
    core c -> batch b = c//2, output-dim half j = c%2.
    Each core computes the full affinity pipeline + splat summary M for its
    batch (duplicated within the pair), and produces out[b][:, j*512:(j+1)*512].

Affinities are computed in [k, s] orientation so that:
  - the xc matmuls keep the centers tile stationary with a 512-wide moving
    operand (few, large PE instructions),
  - the c_sq term rides in as the activation bias (per-partition = per-k),
  - the |x|^2 term enters as a rank-1 matmul accumulation
    (gvec[1,K].T @ xsq_row[1,S]) on top of the same PSUM chunk.
Normalization runs per 512-column chunk so the ACT/DVE/PE stages of
consecutive chunks pipeline.

Matmul operands are bf16; accumulation fp32 in PSUM. The exp() input here is
~-500 which underflows to exactly 0.0 — faithfully matching the reference
numerics (the fp32 reference also underflows; no softmax max-subtraction).
"""
import numpy as np
import ml_dtypes

import concourse.bass as bass
import concourse.bacc as bacc
import concourse.tile as tile
from concourse import mybir
from concourse.masks import make_identity
from concourse.bass_utils import run_bass_kernel_spmd

B, S, D, K = 4, 2048, 1024, 64
P = 128
ST = S // P          # 16 s-tiles
DT = D // P          # 8 d-tiles
NC_CHUNK = 512       # PSUM-bank-sized column chunk
CH = S // NC_CHUNK   # 4 chunks
HALF = D // 2        # 512 output-dim half per core
EPS = 1e-8

BF = mybir.dt.bfloat16
F32 = mybir.dt.float32
BF_NP = ml_dtypes.bfloat16

_CACHE = {}


def build_nc(phase="full"):
    """phase: 'dma' (loads + zero out), 'full'."""
    nc = bacc.Bacc("TRN2", target_bir_lowering=False, debug=False)

    xn_d = nc.dram_tensor("xn", [S, D], BF, kind="ExternalInput")
    xt_d = nc.dram_tensor("xt", [D, S], BF, kind="ExternalInput")
    cts_d = nc.dram_tensor("cts", [D, K], BF, kind="ExternalInput")
    gvec_d = nc.dram_tensor("gvec", [1, K], BF, kind="ExternalInput")
    bcol_d = nc.dram_tensor("bcol", [K, 1], F32, kind="ExternalInput")
    wvt_d = nc.dram_tensor("wvt", [D, D], BF, kind="ExternalInput")
    wot_d = nc.dram_tensor("wot", [D, HALF], BF, kind="ExternalInput")
    out_d = nc.dram_tensor("out", [S, HALF], F32, kind="ExternalOutput")

    with tile.TileContext(nc) as tc:
        with tc.tile_pool(name="persist", bufs=1) as persist:
            # ---- persistent SBUF tensors -------------------------------
            ident = persist.tile([P, P], BF)
            make_identity(nc, ident)
            ones_col = persist.tile([P, 1], BF)
            nc.vector.memset(ones_col[:], 1.0)
            ones_row = persist.tile([1, K], BF)
            nc.vector.memset(ones_row[:], 1.0)

            cts_sb = persist.tile([P, DT, K], BF)
            for dt in range(DT):
                nc.sync.dma_start(
                    out=cts_sb[:, dt, :], in_=cts_d.ap()[dt * P:(dt + 1) * P, :]
                )
            gvec = persist.tile([1, K], BF)
            nc.sync.dma_start(out=gvec[:], in_=gvec_d.ap())
            bcol = persist.tile([K, 1], F32)
            nc.sync.dma_start(out=bcol[:], in_=bcol_d.ap())

            # x in both layouts, fully resident
            xt_sb = persist.tile([P, DT, S], BF)
            for dt in range(DT):
                nc.sync.dma_start(
                    out=xt_sb[:, dt, :], in_=xt_d.ap()[dt * P:(dt + 1) * P, :]
                )
            xn_sb = persist.tile([P, ST, D], BF)
            for st in range(ST):
                nc.sync.dma_start(
                    out=xn_sb[:, st, :], in_=xn_d.ap()[st * P:(st + 1) * P, :]
                )

            # weights, fully resident
            wvt_sb = persist.tile([P, DT, D], BF)
            for dt in range(DT):
                nc.sync.dma_start(
                    out=wvt_sb[:, dt, :], in_=wvt_d.ap()[dt * P:(dt + 1) * P, :]
                )
            wot_sb = persist.tile([P, DT, HALF], BF)
            for dt in range(DT):
                nc.sync.dma_start(
                    out=wot_sb[:, dt, :], in_=wot_d.ap()[dt * P:(dt + 1) * P, :]
                )

            # squares of x^T tiles (for |x|^2 column sums)
            sq_sb = persist.tile([P, DT, S], BF)
            # affinity tensors
            au_bf = persist.tile([K, S], BF)        # exp(..), unnormalized
            A_ksb = persist.tile([K, S], BF)        # normalized affinities
            A_sk = persist.tile([P, ST, K], BF)     # transposed slices
            m_sb = persist.tile([K, D], BF)

            if phase == "dma":
                with tc.tile_pool(name="zo", bufs=2) as zo:
                    for st in range(ST):
                        o_sb = zo.tile([P, HALF], F32, tag="o_sb")
                        nc.vector.memset(o_sb[:], 0.0)
                        nc.sync.dma_start(
                            out=out_d.ap()[st * P:(st + 1) * P, :], in_=o_sb[:],
                        )
            else:
                _emit_main(nc, tc, persist, locals())

    nc.compile()
    return nc


def _emit_main(nc, tc, persist, env):
    ident = env["ident"]; ones_col = env["ones_col"]; ones_row = env["ones_row"]
    cts_sb = env["cts_sb"]; gvec = env["gvec"]; bcol = env["bcol"]
    xt_sb = env["xt_sb"]; xn_sb = env["xn_sb"]
    wvt_sb = env["wvt_sb"]; wot_sb = env["wot_sb"]
    sq_sb = env["sq_sb"]; au_bf = env["au_bf"]; A_ksb = env["A_ksb"]
    A_sk = env["A_sk"]; m_sb = env["m_sb"]; out_d = env["out_d"]

    # ---- phase A: affinities + M -----------------------------------
    # squares first (feed the |x|^2 rank-1 term for every chunk)
    for dt in range(DT):
        nc.vector.tensor_mul(sq_sb[:, dt, :], xt_sb[:, dt, :], xt_sb[:, dt, :])

    with (
        tc.tile_pool(name="ps_M_pool", bufs=1, space="PSUM") as ps_M_pool,
        tc.tile_pool(name="pa_ks", bufs=2, space="PSUM") as pa_ks,
        tc.tile_pool(name="pa_row", bufs=2, space="PSUM") as pa_row,
        tc.tile_pool(name="pa_b", bufs=1, space="PSUM") as pa_b,
        tc.tile_pool(name="pa_tr", bufs=1, space="PSUM") as pa_tr,
        tc.tile_pool(name="pa_sb", bufs=3) as pa_sb,
    ):
        ps_M = ps_M_pool.tile([K, D], F32)

        for c in range(CH):
            cs = slice(c * NC_CHUNK, (c + 1) * NC_CHUNK)

            # |x|^2 row for this chunk: ones^T @ squares
            ps_xsq = pa_row.tile([1, NC_CHUNK], F32, tag="rowps")
            for dt in range(DT):
                nc.tensor.matmul(
                    ps_xsq[:], ones_col[:], sq_sb[:, dt, cs],
                    start=(dt == 0), stop=(dt == DT - 1),
                )
            xsq_row = pa_sb.tile([1, NC_CHUNK], BF, tag="xsq_row")
            nc.scalar.copy(out=xsq_row[:], in_=ps_xsq[:])

            # xc (scaled) + rank-1 |x|^2 term
            ps_ks = pa_ks.tile([K, NC_CHUNK], F32, tag="ps_ks")
            for dt in range(DT):
                nc.tensor.matmul(
                    ps_ks[:], cts_sb[:, dt, :], xt_sb[:, dt, cs],
                    start=(dt == 0), stop=False,
                )
            nc.tensor.matmul(
                ps_ks[:], gvec[:], xsq_row[:], start=False, stop=True,
            )

            # exp with per-k bias (the c_sq term)
            nc.scalar.activation(
                out=au_bf[:, cs], in_=ps_ks[:],
                func=mybir.ActivationFunctionType.Exp,
                bias=bcol[:], scale=1.0,
            )

            # denominator + reciprocal
            ps_den = pa_row.tile([1, NC_CHUNK], F32, tag="rowps")
            nc.tensor.matmul(
                ps_den[:], ones_col[:K, :], au_bf[:, cs],
                start=True, stop=True,
            )
            den_sb = pa_sb.tile([1, NC_CHUNK], F32, tag="den_sb")
            nc.vector.tensor_scalar_add(den_sb[:], ps_den[:], EPS)
            rec_row = pa_sb.tile([1, NC_CHUNK], BF, tag="rec_row")
            with nc.allow_low_precision(reason="bf16 reciprocal row is intended"):
                nc.vector.reciprocal(out=rec_row[:], in_=den_sb[:])

            # broadcast reciprocal over k and normalize
            ps_B = pa_b.tile([K, NC_CHUNK], F32, tag="ps_B")
            nc.tensor.matmul(
                ps_B[:], ones_row[:], rec_row[:], start=True, stop=True,
            )
            nc.vector.tensor_mul(A_ksb[:, cs], au_bf[:, cs], ps_B[:])

            # transposed A slices + M accumulation for this chunk
            for sti in range(NC_CHUNK // P):
                st = c * (NC_CHUNK // P) + sti
                ps_tr = pa_tr.tile([P, K], BF, tag="ps_tr")
                nc.tensor.transpose(
                    ps_tr[:], A_ksb[:, st * P:(st + 1) * P], ident[:K, :K],
                )
                nc.any.tensor_copy(out=A_sk[:, st, :], in_=ps_tr[:])
                for h in range(2):
                    nc.tensor.matmul(
                        ps_M[:, h * 512:(h + 1) * 512],
                        A_sk[:, st, :],
                        xn_sb[:, st, h * 512:(h + 1) * 512],
                        start=(st == 0), stop=(st == ST - 1),
                    )

        # stash M to SBUF before the PSUM pools close
        nc.any.tensor_copy(out=m_sb[:], in_=ps_M[:])

    # ---- phase B: weight chain + output ----------------------------
    with (
        tc.tile_pool(name="pb_sb", bufs=1) as pb_sb,
        tc.tile_pool(name="pb_ps", bufs=1, space="PSUM") as pb_ps,
        tc.tile_pool(name="pb_ptr", bufs=2, space="PSUM") as pb_ptr,
        tc.tile_pool(name="pb_pso", bufs=3, space="PSUM") as pb_pso,
    ):
        # M^T tiles
        mt_sb = persist.tile([P, DT, K], BF)
        for dt in range(DT):
            ps_mt = pb_ptr.tile([P, K], BF, tag="ps_mt")
            nc.tensor.transpose(
                ps_mt[:], m_sb[:, dt * P:(dt + 1) * P], ident[:K, :K],
            )
            nc.any.tensor_copy(out=mt_sb[:, dt, :], in_=ps_mt[:])

        # N = M @ Wv.T
        ps_N = pb_ps.tile([K, D], F32, tag="ps_N")
        for dt in range(DT):
            for h in range(2):
                nc.tensor.matmul(
                    ps_N[:, h * 512:(h + 1) * 512],
                    mt_sb[:, dt, :],
                    wvt_sb[:, dt, h * 512:(h + 1) * 512],
                    start=(dt == 0), stop=(dt == DT - 1),
                )
        n_sb = pb_sb.tile([K, D], BF, tag="n_sb")
        nc.any.tensor_copy(out=n_sb[:], in_=ps_N[:])

        # N^T tiles
        nt_sb = persist.tile([P, DT, K], BF)
        for et in range(DT):
            ps_nt = pb_ptr.tile([P, K], BF, tag="ps_mt")
            nc.tensor.transpose(
                ps_nt[:], n_sb[:, et * P:(et + 1) * P], ident[:K, :K],
            )
            nc.any.tensor_copy(out=nt_sb[:, et, :], in_=ps_nt[:])

        # P = N @ Wo_half.T
        ps_P = pb_ps.tile([K, HALF], F32, tag="ps_P")
        for et in range(DT):
            nc.tensor.matmul(
                ps_P[:], nt_sb[:, et, :], wot_sb[:, et, :],
                start=(et == 0), stop=(et == DT - 1),
            )
        p_sb = pb_sb.tile([K, HALF], BF, tag="p_sb")
        nc.any.tensor_copy(out=p_sb[:], in_=ps_P[:])

        # out[st] = A[st] @ P
        with tc.tile_pool(name="pb_out", bufs=3) as pb_out:
            for st in range(ST):
                ps_o = pb_pso.tile([P, HALF], F32, tag="ps_o")
                nc.tensor.matmul(
                    ps_o[:], A_ksb[:, st * P:(st + 1) * P], p_sb[:],
                    start=True, stop=True,
                )
                o_sb = pb_out.tile([P, HALF], F32, tag="o_sb")
                nc.any.tensor_copy(out=o_sb[:], in_=ps_o[:])
                nc.sync.dma_start(
                    out=out_d.ap()[st * P:(st + 1) * P, :], in_=o_sb[:],
                )


def _host_prep(x, splat_centers, splat_log_scales, w_value, w_out):
    """Fold scales into weights; build per-core input maps."""
    x = np.asarray(x, dtype=np.float32)
    centers = np.asarray(splat_centers, dtype=np.float32)
    log_scales = np.asarray(splat_log_scales, dtype=np.float32)
    w_value = np.asarray(w_value, dtype=np.float32)
    w_out = np.asarray(w_out, dtype=np.float32)

    scales = np.clip(np.exp(log_scales), 0.1, 2.0)
    inv_ss = (1.0 / (scales * scales)).astype(np.float32)          # [K]
    cts = (centers.T * inv_ss[None, :]).astype(BF_NP)              # [D,K]
    c_sq = (centers * centers).sum(axis=1).astype(np.float32)      # [K]
    bcol = (-0.5 * c_sq * inv_ss)[:, None].astype(np.float32)      # [K,1]
    gvec = (-0.5 * inv_ss)[None, :].astype(BF_NP)                  # [1,K]
    wvt = w_value.T.astype(BF_NP).copy()                           # [D,D]

    in_maps = []
    for c in range(8):
        b, j = divmod(c, 2)
        xb = x[b]
        in_maps.append({
            "xn": xb.astype(BF_NP),
            "xt": xb.T.astype(BF_NP).copy(),
            "cts": cts,
            "gvec": gvec,
            "bcol": bcol,
            "wvt": wvt,
            "wot": w_out[j * HALF:(j + 1) * HALF, :].T.astype(BF_NP).copy(),
        })
    return in_maps


def run_on_hw(in_maps, trace=False, phase="full"):
    key = f"nc_{phase}"
    if key not in _CACHE:
        _CACHE[key] = build_nc(phase)
    return run_bass_kernel_spmd(_CACHE[key], in_maps, list(range(8)), trace=trace)


def kernel(**inputs) -> np.ndarray:
    in_maps = _host_prep(**inputs)
    res = run_on_hw(in_maps)
    out = np.empty((B, S, D), dtype=np.float32)
    for c in range(8):
        b, j = divmod(c, 2)
        out[b][:, j * HALF:(j + 1) * HALF] = res.results[c]["out"]
    return out
